# revision 1
# baseline (speedup 1.0000x reference)
"""AlphaEntmax attention (alpha=1.5) on 8 Trainium2 NeuronCores.

Sharding: batch*heads data-parallel. 32 (b,h) pairs -> 8 cores, 4 heads each
(cores 0-3: batch 0, cores 4-7: batch 1). Each core computes q/k/v projections
for its 4 heads, causal scores, entmax via Newton active-set iteration
(alpha=1.5 => p = relu(Xs - tau)^2, solve sum p = 1 for tau), p@v, and its
partial contribution to the output projection. Host sums partials + bias.

entmax equivalence: the reference runs 50 fp32 bisection steps, but fp32
bisection is stationary after ~25 steps, so any solver that reaches the same
fixed point matches it. Newton on f(tau) = sum relu(Xs-tau)^2 - 1 (convex,
decreasing, monotone from tau0 = sampled rowmax - 1) converges in 8 updates;
the first updates evaluate f on a column-prefix subsample (per-row target
fraction tiles fr4/fr2) since early steps only need coarse stats. A 9th
stats-only sweep yields the normalizer sum(p). The final p is recomputed in
transposed layout (S^T via one K=65 matmul that folds ntau in as an extra
contraction row) so p^T feeds p@v directly without PE transposes.
End-to-end absmax error vs the reference: ~1.6e-4 relative (verified).
"""

from contextlib import ExitStack

import numpy as np

C = 2048          # sequence length
E = 1024          # embed dim
HLOC = 4          # heads per core
HS = 64           # head size
DH = HLOC * HS    # 256 per-core projection width
NEWTON_K = 8  # retained for reference; SCHED drives the sweep count
NEG = np.float32(-1.0e30)
QI_GROUP = 4      # q-tiles processed per Newton group
SCHED = [4, 4, 2, 2, 1, 1, 1, 1]  # per-update column subsample divisor
DEBUG_TAPS = False

_NC_CACHE = {}


def _build_nc():
    import concourse.bacc as bacc
    import concourse.mybir as mybir
    import concourse.tile as tile
    from concourse.masks import make_identity

    F32 = mybir.dt.float32
    ALU = mybir.AluOpType
    AFT = mybir.ActivationFunctionType
    AX = mybir.AxisListType

    nc = bacc.Bacc("TRN2", target_bir_lowering=False, debug=False, num_devices=8)

    xt = nc.dram_tensor("xt", [E, C], F32, kind="ExternalInput")
    wq = nc.dram_tensor("wq", [E, DH], F32, kind="ExternalInput")
    wk = nc.dram_tensor("wk", [E, DH], F32, kind="ExternalInput")
    wv = nc.dram_tensor("wv", [E, DH], F32, kind="ExternalInput")
    bqd = nc.dram_tensor("bq", [1, DH], F32, kind="ExternalInput")
    bkd = nc.dram_tensor("bk", [1, DH], F32, kind="ExternalInput")
    bvd = nc.dram_tensor("bv", [1, DH], F32, kind="ExternalInput")
    wu = nc.dram_tensor("wu", [DH, E], F32, kind="ExternalInput")
    mbd = nc.dram_tensor("mb", [128, 128], F32, kind="ExternalInput")
    mbtd = nc.dram_tensor("mbt", [128, 128], F32, kind="ExternalInput")
    fr4d = nc.dram_tensor("fr4", [128, 16], F32, kind="ExternalInput")
    fr2d = nc.dram_tensor("fr2", [128, 16], F32, kind="ExternalInput")
    out = nc.dram_tensor("out", [C, E], F32, kind="ExternalOutput")
    if DEBUG_TAPS:
        ohdbg = nc.dram_tensor("ohdbg", [C, DH], F32, kind="ExternalOutput")
        ntdbg = nc.dram_tensor("ntdbg", [HLOC, C], F32, kind="ExternalOutput")
        ptdbg = nc.dram_tensor("ptdbg", [128, C], F32, kind="ExternalOutput")

    NQT = C // 128  # 16 q tiles

    with tile.TileContext(nc) as tc, ExitStack() as ctx:
        const = ctx.enter_context(tc.tile_pool(name="const", bufs=1))
        pers = ctx.enter_context(tc.tile_pool(name="pers", bufs=1))

        ident = const.tile([128, 128], F32, tag="ident", name="ident")
        make_identity(nc, ident[:])
        ones = const.tile([1, 512], F32, tag="ones", name="ones")
        nc.vector.memset(ones[:], 1.0)
        mb = const.tile([128, 128], F32, tag="mb", name="mb")
        nc.sync.dma_start(mb[:], mbd[:])
        mbt = const.tile([128, 128], F32, tag="mbt", name="mbt")
        nc.sync.dma_start(mbt[:], mbtd[:])
        fr = {}
        for dv, frd in ((4, fr4d), (2, fr2d)):
            fr[dv] = const.tile([128, 16], F32, tag=f"fr{dv}", name=f"fr{dv}")
            nc.sync.dma_start(fr[dv][:], frd[:])
        wu_t = [const.tile([128, E], F32, tag=f"wu{i}", name=f"wu{i}") for i in range(2)]
        for i in range(2):
            nc.sync.dma_start(wu_t[i][:], wu[128 * i:128 * (i + 1), :])

        # persistent activation tensors
        qT = [pers.tile([128, C], F32, tag=f"qT{i}", name=f"qT{i}") for i in range(2)]
        kT = [pers.tile([128, C], F32, tag=f"kT{i}", name=f"kT{i}") for i in range(2)]
        vt = [pers.tile([128, DH], F32, tag=f"vt{i}", name=f"vt{i}") for i in range(NQT)]
        # head outputs in [d, q] layout; pairs packed two per tile in
        # partition halves (rows 0-63 = even pair, 64-127 = odd pair)
        ohd = [pers.tile([128, C], F32, tag=f"ohd{i}", name=f"ohd{i}")
               for i in range(2)]
        # augmented operands for the S^T pass: row 64 of kaug is ones, row 64
        # of qaug is ntau, so one K=65 matmul computes S^T + ntau along q.
        kaug = pers.tile([65, C], F32, tag="kaug", name="kaug")
        qaug = pers.tile([65, C], F32, tag="qaug", name="qaug")
        nc.vector.memset(kaug[64:65, :], 1.0)

        # ---- phase P: projections ----
        with ExitStack() as pctx:
            wpool = pctx.enter_context(tc.tile_pool(name="wpool", bufs=1))
            ppsum = pctx.enter_context(
                tc.tile_pool(name="ppsum", bufs=2, space="PSUM"))

            xt_t = [wpool.tile([128, C], F32, tag=f"xt{i}", name=f"xt{i}") for i in range(8)]
            for i in range(8):
                nc.sync.dma_start(xt_t[i][:], xt[128 * i:128 * (i + 1), :])

            wtiles = {}
            btiles = {}
            for name, wd, bd in (("q", wq, bqd), ("k", wk, bkd), ("v", wv, bvd)):
                wtiles[name] = [wpool.tile([128, DH], F32, tag=f"w{name}{i}", name=f"w{name}{i}")
                                for i in range(8)]
                for i in range(8):
                    nc.sync.dma_start(wtiles[name][i][:],
                                      wd[128 * i:128 * (i + 1), :])
                btiles[name] = wpool.tile([1, DH], F32, tag=f"b{name}", name=f"b{name}")
                nc.sync.dma_start(btiles[name][:], bd[:])

            # qT/kT in [dh, t] layout: psum [128dh, 512t]
            for name, dst in (("q", qT), ("k", kT)):
                for half in range(2):
                    hsl = slice(128 * half, 128 * (half + 1))
                    for chk in range(C // 512):
                        csl = slice(512 * chk, 512 * (chk + 1))
                        ps = ppsum.tile([128, 512], F32, tag="pqk", name="pqk")
                        nc.tensor.matmul(ps[:], btiles[name][:, hsl], ones[:],
                                         start=True, stop=False)
                        for ei in range(8):
                            nc.tensor.matmul(ps[:], wtiles[name][ei][:, hsl],
                                             xt_t[ei][:, csl],
                                             start=False, stop=(ei == 7))
                        nc.vector.tensor_copy(dst[half][:, csl], ps[:])

            # v in [t, dh] layout: psum [128t, 256dh]
            for ti in range(NQT):
                tsl = slice(128 * ti, 128 * (ti + 1))
                ps = ppsum.tile([128, DH], F32, tag="pv", name="pv")
                nc.tensor.matmul(ps[:], ones[:, :128], btiles["v"][:],
                                 start=True, stop=False)
                for ei in range(8):
                    nc.tensor.matmul(ps[:], xt_t[ei][:, tsl], wtiles["v"][ei][:],
                                     start=False, stop=(ei == 7))
                nc.vector.tensor_copy(vt[ti][:], ps[:])

        # ---- phase A: per-(b,h) attention ----
        xs_pool = ctx.enter_context(tc.tile_pool(name="xs", bufs=QI_GROUP))
        u_pool = ctx.enter_context(tc.tile_pool(name="u", bufs=2))
        p_pool = ctx.enter_context(tc.tile_pool(name="p", bufs=1))
        pt_pool = ctx.enter_context(tc.tile_pool(name="pt", bufs=2))
        ptk_pool = ctx.enter_context(tc.tile_pool(name="ptk", bufs=4))
        ut_pool = ctx.enter_context(tc.tile_pool(name="ut", bufs=2))
        st_pool = ctx.enter_context(tc.tile_pool(name="st", bufs=3))
        sc_psum = ctx.enter_context(tc.tile_pool(name="scp", bufs=2, space="PSUM"))
        tr_psum = ctx.enter_context(tc.tile_pool(name="trp", bufs=1, space="PSUM"))
        pv_psum = ctx.enter_context(tc.tile_pool(name="pvp", bufs=2, space="PSUM"))

        def emit_newton(pair):
            qTh = qT[pair // 2][64 * (pair % 2):64 * (pair % 2) + 64, :]
            kTh = kT[pair // 2][64 * (pair % 2):64 * (pair % 2) + 64, :]
            mx = st_pool.tile([128, NQT], F32, tag="mx", name="mx")
            tau = st_pool.tile([128, NQT], F32, tag="tau", name="tau")
            Lt = st_pool.tile([128, NQT], F32, tag="Lt", name="Lt")
            Qt = st_pool.tile([128, NQT], F32, tag="Qt", name="Qt")
            d1 = st_pool.tile([128, NQT], F32, tag="d1", name="d1")
            d2 = st_pool.tile([128, NQT], F32, tag="d2", name="d2")
            Sp = st_pool.tile([128, NQT], F32, tag="Sp", name="Sp")
            rS = st_pool.tile([128, NQT], F32, tag="rS", name="rS")

            for g0 in range(0, NQT, QI_GROUP):
                gqis = range(g0, min(g0 + QI_GROUP, NQT))
                xs_g = {}
                # scores + rowmax + tau0
                for qi in gqis:
                    ncol = 128 * (qi + 1)
                    xs = xs_pool.tile([128, C], F32, tag="xs", name="xs")
                    xs_g[qi] = xs
                    nchunk = (ncol + 511) // 512
                    for ch in range(nchunk):
                        c0 = 512 * ch
                        w = min(512, ncol - c0)
                        has_diag = (c0 + w == ncol)
                        ps = sc_psum.tile([128, 512], F32, tag="sc", name="sc")
                        nc.tensor.matmul(ps[:, :w], qTh[:, 128 * qi:128 * (qi + 1)],
                                         kTh[:, c0:c0 + w],
                                         start=True, stop=not has_diag)
                        if has_diag:
                            nc.tensor.matmul(ps[:, w - 128:w], ident[:], mb[:],
                                             start=False, stop=True)
                        nc.scalar.activation(xs[:, c0:c0 + w], ps[:, :w],
                                             AFT.Copy)
                    # sampled row max (::2): underestimates => tau0 still a
                    # valid Newton-from-below start
                    nc.vector.reduce_max(mx[:, qi:qi + 1],
                                         xs[:, 0:ncol:2], axis=AX.X)
                    # ntau = -(max - 1) = 1 - max
                    nc.vector.tensor_scalar(out=tau[:, qi:qi + 1],
                                            in0=mx[:, qi:qi + 1],
                                            scalar1=1.0, scalar2=-1.0,
                                            op0=ALU.subtract, op1=ALU.mult)

                gsl = slice(g0, g0 + len(list(gqis)))
                # Newton refreshes; tau tile holds ntau = -tau. The last
                # refresh only evaluates stats (no update): its Q = sum p at
                # the final tau, i.e. the normalizer.
                for it in range(len(SCHED) + 1):
                    div = SCHED[it] if it < len(SCHED) else 1
                    for qi in gqis:
                        ncol = 128 * (qi + 1)
                        w = max(128, ncol // div)
                        u = u_pool.tile([128, C], F32, tag="u", name="u")
                        p = p_pool.tile([128, C], F32, tag="p", name="p")
                        # ACT: u = relu(xs + ntau), L = sum u
                        nc.scalar.activation(u[:, :w], xs_g[qi][:, :w],
                                             AFT.Relu, bias=tau[:, qi:qi + 1],
                                             accum_out=Lt[:, qi:qi + 1])
                        # DVE: p = (xs + ntau) * u = relu^2, Q = sum p
                        nc.vector.scalar_tensor_tensor(
                            out=p[:, :w], in0=xs_g[qi][:, :w],
                            scalar=tau[:, qi:qi + 1], in1=u[:, :w],
                            op0=ALU.add, op1=ALU.mult,
                            accum_out=Qt[:, qi:qi + 1])
                    if it == len(SCHED):
                        break
                    # delta = (Q - frac)*0.5*(1/L); ntau -= delta
                    if div > 1:
                        nc.vector.tensor_tensor(out=d1[:, gsl], in0=Qt[:, gsl],
                                                in1=fr[div][:, gsl],
                                                op=ALU.subtract)
                        nc.vector.tensor_scalar(out=d1[:, gsl], in0=d1[:, gsl],
                                                scalar1=0.5, scalar2=None,
                                                op0=ALU.mult)
                    else:
                        nc.vector.tensor_scalar(out=d1[:, gsl], in0=Qt[:, gsl],
                                                scalar1=1.0, scalar2=0.5,
                                                op0=ALU.subtract, op1=ALU.mult)
                    nc.vector.reciprocal(d2[:, gsl], Lt[:, gsl])
                    nc.vector.tensor_tensor(out=d1[:, gsl], in0=d1[:, gsl],
                                            in1=d2[:, gsl], op=ALU.mult)
                    nc.vector.tensor_tensor(out=tau[:, gsl], in0=tau[:, gsl],
                                            in1=d1[:, gsl], op=ALU.subtract)
                # 1/sum(p) for this group's columns
                nc.vector.reciprocal(rS[:, gsl], Qt[:, gsl])
            return tau, rS

        def emit_st(pair, tau, rS):
            qTh = qT[pair // 2][64 * (pair % 2):64 * (pair % 2) + 64, :]
            kTh = kT[pair // 2][64 * (pair % 2):64 * (pair % 2) + 64, :]
            # Stage the pair's S^T operands: copy kTh/qTh into the augmented
            # tiles; ntau goes to qaug row 64 in q-row order (PE-transpose
            # [128,16] -> [16,128] psum, copy to SBUF, DMA flatten).
            nc.vector.tensor_copy(kaug[0:64, :], kTh)
            nc.vector.tensor_copy(qaug[0:64, :], qTh)
            ntp = tr_psum.tile([128, 512], F32, tag="tr", name="tr")
            nc.tensor.transpose(ntp[:NQT, :128], tau[:, :NQT], ident[:])
            ntT = pt_pool.tile([NQT, 128], F32, tag="ntT", name="ntT")
            nc.vector.tensor_copy(ntT[:], ntp[:NQT, :128])
            for qi in range(NQT):
                nc.sync.dma_start(qaug[64:65, 128 * qi:128 * (qi + 1)],
                                  ntT[qi:qi + 1, :])
            if DEBUG_TAPS:
                nc.sync.dma_start(ntdbg[pair:pair + 1, :], qaug[64:65, :])
            # rS to row layout [1, C] for the PE broadcast scale
            ntp2 = tr_psum.tile([128, 512], F32, tag="tr", name="tr")
            nc.tensor.transpose(ntp2[:NQT, :128], rS[:, :NQT], ident[:])
            rT = pt_pool.tile([NQT, 128], F32, tag="rT", name="rT")
            nc.vector.tensor_copy(rT[:], ntp2[:NQT, :128])
            rSrow = pt_pool.tile([1, C], F32, tag="rSrow", name="rSrow")
            for qi in range(NQT):
                nc.sync.dma_start(rSrow[0:1, 128 * qi:128 * (qi + 1)],
                                  rT[qi:qi + 1, :])

            # S^T pass: recompute scores transposed (k on partitions), with
            # ntau folded in via the K=65 augmented operands, mask the
            # diagonal block, then pT = relu(.)^2 via DVE relu + ACT square.
            # p@v computes out^T = vt^T @ pT in 512-wide q-chunks, blocks of
            # 4 k-tiles (contiguous psum groups; interleaved groups are
            # illegal), accumulated into ohd in SBUF.
            KTB = 4
            for ktb in range(NQT // KTB):
                ptks = {}
                for kt in range(KTB * ktb, KTB * (ktb + 1)):
                    q_lo = 128 * kt
                    ptk = ptk_pool.tile([128, C], F32, tag="ptk", name="ptk")
                    ptks[kt] = ptk
                    for j in range(q_lo // 512, 4):
                        q0 = max(512 * j, q_lo)
                        w = 512 * (j + 1) - q0
                        has_diag = (q0 == q_lo)
                        ps = sc_psum.tile([128, 512], F32, tag="sc", name="sc")
                        nc.tensor.matmul(ps[:, :w],
                                         kaug[:, q_lo:q_lo + 128],
                                         qaug[:, q0:q0 + w],
                                         start=True, stop=not has_diag)
                        if has_diag:
                            # diag block: strict lower-tri mask in S^T layout
                            nc.tensor.matmul(ps[:, :128], ident[:], mbt[:],
                                             start=False, stop=True)
                        ut = ut_pool.tile([128, 512], F32, tag="ut", name="ut")
                        nc.vector.tensor_scalar(out=ut[:, :w], in0=ps[:, :w],
                                                scalar1=0.0, scalar2=None,
                                                op0=ALU.max)
                        nc.scalar.activation(ptk[:, q0:q0 + w], ut[:, :w],
                                             AFT.Square)
                    if DEBUG_TAPS and pair == 0 and kt == 0:
                        nc.sync.dma_start(ptdbg[:, :], ptk[:, :])
                p2 = 64 * (pair % 2)
                od = ohd[pair // 2]
                for j in range(4):
                    kts = [kt for kt in range(KTB * ktb, KTB * (ktb + 1))
                           if 128 * kt < 512 * (j + 1)]
                    if not kts:
                        continue
                    a0 = max(512 * j, 128 * kts[0])
                    po = pv_psum.tile([128, 512], F32, tag="po", name="po",
                                      bufs=3)
                    for i, kt in enumerate(kts):
                        ak = max(512 * j, 128 * kt)
                        nc.tensor.matmul(
                            po[p2:p2 + 64, ak - 512 * j:512],
                            vt[kt][:, HS * pair:HS * (pair + 1)],
                            ptks[kt][:, ak:512 * (j + 1)],
                            start=(i == 0), stop=(i == len(kts) - 1))
                    if ktb == 0:
                        nc.vector.tensor_copy(
                            od[p2:p2 + 64, 512 * j:512 * (j + 1)],
                            po[p2:p2 + 64, :])
                    else:
                        nc.vector.tensor_tensor(
                            out=od[p2:p2 + 64, a0:512 * (j + 1)],
                            in0=od[p2:p2 + 64, a0:512 * (j + 1)],
                            in1=po[p2:p2 + 64, a0 - 512 * j:512], op=ALU.add)
            # scale columns by 1/sum(p) via PE broadcast of the rS row
            for j in range(4):
                rb = pv_psum.tile([128, 512], F32, tag="rb", name="rb")
                nc.tensor.matmul(rb[p2:p2 + 64, :], ones[0:1, :64],
                                 rSrow[0:1, 512 * j:512 * (j + 1)],
                                 start=True, stop=True)
                nc.vector.tensor_tensor(
                    out=od[p2:p2 + 64, 512 * j:512 * (j + 1)],
                    in0=od[p2:p2 + 64, 512 * j:512 * (j + 1)],
                    in1=rb[p2:p2 + 64, :], op=ALU.mult)

        # software pipeline: Newton(p+1) (DVE/ACT heavy) is emitted before
        # S^T(p) consumers drain, so it overlaps the PE-heavy S^T phase.
        state = {}
        state[0] = emit_newton(0)
        for p in range(HLOC):
            if p + 1 < HLOC:
                state[p + 1] = emit_newton(p + 1)
            emit_st(p, *state.pop(p))

        # ---- phase O: output projection ----
        for qi in range(NQT):
            for ch in range(2):
                csl = slice(512 * ch, 512 * (ch + 1))
                ps = sc_psum.tile([128, 512], F32, tag="sc", name="sc")
                for i in range(2):
                    nc.tensor.matmul(ps[:],
                                     ohd[i][:, 128 * qi:128 * (qi + 1)],
                                     wu_t[i][:, csl], start=(i == 0),
                                     stop=(i == 1))
                osb = pt_pool.tile([128, 512], F32, tag="osb", name="osb")
                nc.scalar.activation(osb[:], ps[:], AFT.Copy)
                nc.sync.dma_start(out[128 * qi:128 * (qi + 1), csl], osb[:])

    nc.compile()
    return nc


def _get_nc():
    if "nc" not in _NC_CACHE:
        _NC_CACHE["nc"] = _build_nc()
    return _NC_CACHE["nc"]


def _entmax_bisect_np(X, alpha, n_iter=50):
    d = X.shape[-1]
    am1 = alpha - 1.0
    Xs = (X * am1).astype(np.float32)
    max_val = Xs.max(-1, keepdims=True)
    tau_lo = max_val - np.float32(1.0)
    tau_hi = max_val - np.float32((1.0 / d) ** (1.0 / am1))
    f_lo = (np.clip(Xs - tau_lo, 0, None) ** (1.0 / am1)).sum(-1, keepdims=True) - 1.0
    dm = tau_hi - tau_lo
    p_m = np.zeros_like(Xs)
    for _ in range(n_iter):
        dm = dm * 0.5
        tau_m = tau_lo + dm
        p_m = np.clip(Xs - tau_m, 0, None) ** (1.0 / am1)
        f_m = p_m.sum(-1, keepdims=True) - 1.0
        tau_lo = np.where(f_m * f_lo >= 0, tau_m, tau_lo)
    return p_m / p_m.sum(-1, keepdims=True)


def _numpy_fallback(x, mask, H, hs, alpha, Wq, bq, Wk, bk, Wv, bv, Wu, bu):
    b, c, e = x.shape
    q = (x @ Wq + bq).reshape(b, c, H, hs)
    k = (x @ Wk + bk).reshape(b, c, H, hs)
    v = (x @ Wv + bv).reshape(b, c, H, hs)
    dot = np.einsum('bqhd,bkhd->bhqk', q, k).astype(np.float32) / np.sqrt(hs)
    dot = np.where(mask[:, None], dot, -np.inf).astype(np.float32)
    p = _entmax_bisect_np(dot, float(alpha))
    o = np.einsum('bhqk,bkhd->bqhd', p, v).reshape(b, c, H * hs)
    return (o @ Wu + bu).astype(np.float32)


def make_in_maps(x, alpha_f, hs, Wq, bq, Wk, bk, Wv, bv, Wu):
    s = np.float32((alpha_f - 1.0) / np.sqrt(hs))  # fold entmax+attn scale into q
    mbias = np.triu(np.full((128, 128), NEG, np.float32), 1)
    mbiast = np.ascontiguousarray(mbias.T)
    # per-row sampled-sweep target fractions: row r of q-tile qi has
    # valid = 128*qi + r + 1 columns; a div-subsampled sweep sees the first
    # max(128, ncol/div) of them
    r = np.arange(128)[:, None]
    qi = np.arange(16)[None, :]
    valid = 128 * qi + r + 1
    frs = {}
    for dv in (2, 4):
        w = np.maximum(128, (128 * (qi + 1)) // dv)
        frs[dv] = (np.minimum(w, valid) / valid).astype(np.float32)
    in_maps = []
    for core in range(8):
        bb = core // 4
        hsl = slice((core % 4) * DH, (core % 4) * DH + DH)
        in_maps.append({
            "xt": np.ascontiguousarray(x[bb].T.astype(np.float32)),
            "wq": np.ascontiguousarray(Wq[:, hsl]).astype(np.float32) * s,
            "bq": (bq[hsl] * s).reshape(1, DH).astype(np.float32),
            "wk": np.ascontiguousarray(Wk[:, hsl]).astype(np.float32),
            "bk": bk[hsl].reshape(1, DH).astype(np.float32),
            "wv": np.ascontiguousarray(Wv[:, hsl]).astype(np.float32),
            "bv": bv[hsl].reshape(1, DH).astype(np.float32),
            "wu": np.ascontiguousarray(Wu[hsl, :]).astype(np.float32),
            "mb": mbias,
            "mbt": mbiast,
            "fr4": frs[4],
            "fr2": frs[2],
        })
    return in_maps


def kernel(x, attention_mask, num_heads, head_size, alpha,
           Wq, bq, Wk, bk, Wv, bv, Wu, bu):
    x = np.asarray(x, np.float32)
    mask = np.asarray(attention_mask)
    H = int(num_heads)
    hs = int(head_size)
    alpha_f = float(np.asarray(alpha))
    Wq = np.asarray(Wq, np.float32); bq = np.asarray(bq, np.float32)
    Wk = np.asarray(Wk, np.float32); bk = np.asarray(bk, np.float32)
    Wv = np.asarray(Wv, np.float32); bv = np.asarray(bv, np.float32)
    Wu = np.asarray(Wu, np.float32); bu = np.asarray(bu, np.float32)
    b, c, e = x.shape

    causal = np.tril(np.ones((c, c), dtype=bool))
    supported = (
        (b, c, e, H, hs) == (2, C, E, 16, HS)
        and abs(alpha_f - 1.5) < 1e-6
        and all(np.array_equal(mask[i], causal) for i in range(b))
    )
    if not supported:
        return _numpy_fallback(x, mask, H, hs, alpha_f,
                               Wq, bq, Wk, bk, Wv, bv, Wu, bu)

    from concourse.bass_utils import run_bass_kernel_spmd

    nc = _get_nc()
    in_maps = make_in_maps(x, alpha_f, hs, Wq, bq, Wk, bk, Wv, bv, Wu)
    res = run_bass_kernel_spmd(nc, in_maps, core_ids=list(range(8)))
    o = [res.results[i]["out"] for i in range(8)]
    full = np.stack([o[0] + o[1] + o[2] + o[3],
                     o[4] + o[5] + o[6] + o[7]]) + bu
    return full.astype(np.float32)



# revision 18
# speedup vs baseline: 1.5929x; 1.5929x over previous
"""AlphaEntmax attention (alpha=1.5) on 8 Trainium2 NeuronCores.

Sharding: batch*heads data-parallel. 32 (b,h) pairs -> 8 cores, 4 heads each
(cores 0-3: batch 0, cores 4-7: batch 1). Each core computes q/k/v projections
for its 4 heads, causal scores, entmax via Newton iteration, p@v, and its
partial contribution to the output projection. Host sums partials + bias.

v2 design vs the fp32 baseline:
- all matmuls fp16 (1 cyc/row on PE vs 4 for fp32).
- the PSUM->SBUF score copy is fused with the first relu: u0 = relu(xs +
  ntau0) stored fp16. Entries clipped at tau0 stay clipped for all later
  (larger) tau, so sweeps compute relu(u0 + delta) exactly - in fp16, which
  unlocks the DVE 4x mode for pass A (tensor_scalar add+max, accum L).
- pass B (square, accum Q) split between ACT (Square) and DVE (STT mult)
  per tile, large tiles on ACT.
- Newton schedule: full-width first update from the copy-pass stats, then
  prefix-subsampled sweeps SCHED (fr-corrected), two final full sweeps.
- q/k stored per head as [65, C] fp16: rows 0-64 = head data, row 64 =
  ones (k) / ntau (q), so one K=65 matmul recomputes S^T with ntau folded.
- normalizer sum(p) folded into p@v via a ones column in vt (M=65 matmul);
  output scaled by 1/S via PE row-broadcast.
- two heads' Newton interleaved to hide the per-sweep update latency.
"""

from contextlib import ExitStack

import numpy as np

C = 2048          # sequence length
E = 1024          # embed dim
HLOC = 4          # heads per core
HS = 64           # head size
DH = HLOC * HS    # 256 per-core projection width
NEG = np.float32(-60000.0)   # fp16-safe mask value
SCHED = [16, 16, 8, 8, 4, 2, 1, 1, 1]   # per-sweep prefix divisor
FRDIVS = sorted({d for d in SCHED if d > 1})
# first qi whose pass A runs on ACT (Relu+accum), per divisor; below: DVE
ACT_START = {1: 6, 2: 10, 4: 12, 8: 16, 16: 16}
NQT = C // 128    # 16 q tiles
DEBUG_TAPS = False

_NC_CACHE = {}


def _build_nc():
    import concourse.bacc as bacc
    import concourse.mybir as mybir
    import concourse.tile as tile
    from concourse.masks import make_identity

    F32 = mybir.dt.float32
    F16 = mybir.dt.float16
    ALU = mybir.AluOpType
    AFT = mybir.ActivationFunctionType
    AX = mybir.AxisListType

    nc = bacc.Bacc("TRN2", target_bir_lowering=False, debug=False, num_devices=8)

    xt = nc.dram_tensor("xt", [E, C], F16, kind="ExternalInput")
    wq = nc.dram_tensor("wq", [E, DH], F16, kind="ExternalInput")
    wk = nc.dram_tensor("wk", [E, DH], F16, kind="ExternalInput")
    wv = nc.dram_tensor("wv", [E, DH], F16, kind="ExternalInput")
    bqd = nc.dram_tensor("bq", [1, DH], F16, kind="ExternalInput")
    bkd = nc.dram_tensor("bk", [1, DH], F16, kind="ExternalInput")
    bvd = nc.dram_tensor("bv", [1, DH], F16, kind="ExternalInput")
    wu = nc.dram_tensor("wu", [DH, E], F16, kind="ExternalInput")
    mbd = nc.dram_tensor("mb", [128, 128], F16, kind="ExternalInput")
    mbtd = nc.dram_tensor("mbt", [128, 128], F16, kind="ExternalInput")
    frd = {dv: nc.dram_tensor(f"fr{dv}", [128, NQT], F32, kind="ExternalInput")
           for dv in FRDIVS}
    frc0d = nc.dram_tensor("frc0", [128, NQT], F32, kind="ExternalInput")
    out = nc.dram_tensor("out", [C, E], F32, kind="ExternalOutput")
    if DEBUG_TAPS:
        dbg_ntau = nc.dram_tensor("dbg_ntau", [128, NQT], F32, kind="ExternalOutput")
        dbg_u0 = nc.dram_tensor("dbg_u0", [128, C], F32, kind="ExternalOutput")
        dbg_qaug = nc.dram_tensor("dbg_qaug", [65, C], F32, kind="ExternalOutput")
        dbg_od = nc.dram_tensor("dbg_od", [128, C], F32, kind="ExternalOutput")
        dbg_ptk = nc.dram_tensor("dbg_ptk", [128, C], F32, kind="ExternalOutput")
        dbg_lq = nc.dram_tensor("dbg_lq", [128, 3 * NQT], F32, kind="ExternalOutput")

    with tile.TileContext(nc) as tc, ExitStack() as ctx:
        const = ctx.enter_context(tc.tile_pool(name="const", bufs=1))
        pers = ctx.enter_context(tc.tile_pool(name="pers", bufs=1))

        ident = const.tile([128, 128], F32, tag="ident", name="ident")
        make_identity(nc, ident[:])
        ident16 = const.tile([128, 128], F16, tag="ident16", name="ident16")
        make_identity(nc, ident16[:])
        ones = const.tile([1, 512], F16, tag="ones", name="ones")
        nc.vector.memset(ones[:], 1.0)
        mb = const.tile([128, 128], F16, tag="mb", name="mb")
        nc.sync.dma_start(mb[:], mbd[:])
        mbt = const.tile([128, 128], F16, tag="mbt", name="mbt")
        nc.sync.dma_start(mbt[:], mbtd[:])
        fr = {}
        for dv in FRDIVS:
            fr[dv] = const.tile([128, NQT], F32, tag=f"fr{dv}", name=f"fr{dv}")
            nc.sync.dma_start(fr[dv][:], frd[dv][:])
        frc0 = const.tile([128, NQT], F32, tag="frc0", name="frc0")
        nc.sync.dma_start(frc0[:], frc0d[:])
        wu_t = [const.tile([128, E], F16, tag=f"wu{i}", name=f"wu{i}") for i in range(2)]
        for i in range(2):
            nc.sync.dma_start(wu_t[i][:], wu[128 * i:128 * (i + 1), :])

        # persistent per-head q/k: rows 0-63 head data, row 64 aug (ones/ntau)
        qh = [pers.tile([65, C], F16, tag=f"qh{h}", name=f"qh{h}") for h in range(HLOC)]
        kh = [pers.tile([65, C], F16, tag=f"kh{h}", name=f"kh{h}") for h in range(HLOC)]
        for h in range(HLOC):
            nc.vector.memset(kh[h][64:65, :], 1.0)
        # v in [t, 65*h + d] layout; col 65h+64 = ones (normalizer fold)
        vt = [pers.tile([128, 65 * HLOC], F16, tag=f"vt{i}", name=f"vt{i}")
              for i in range(NQT)]
        for i in range(NQT):
            nc.vector.memset(vt[i][:, 64:65 * HLOC:65], 1.0)
        # head outputs [dh, q], pairs packed two per tile
        ohd = [pers.tile([128, C], F16, tag=f"ohd{i}", name=f"ohd{i}")
               for i in range(2)]

        # ---- phase P: projections (all fp16) ----
        with ExitStack() as pctx:
            wpool = pctx.enter_context(tc.tile_pool(name="wpool", bufs=1))
            ppsum = pctx.enter_context(
                tc.tile_pool(name="ppsum", bufs=2, space="PSUM"))

            xt_t = [wpool.tile([128, C], F16, tag=f"xt{i}", name=f"xt{i}") for i in range(8)]
            for i in range(8):
                nc.sync.dma_start(xt_t[i][:], xt[128 * i:128 * (i + 1), :])

            wtiles = {}
            btiles = {}
            for name, wd, bd in (("q", wq, bqd), ("k", wk, bkd), ("v", wv, bvd)):
                wtiles[name] = [wpool.tile([128, DH], F16, tag=f"w{name}{i}", name=f"w{name}{i}")
                                for i in range(8)]
                for i in range(8):
                    nc.sync.dma_start(wtiles[name][i][:],
                                      wd[128 * i:128 * (i + 1), :])
                btiles[name] = wpool.tile([1, DH], F16, tag=f"b{name}", name=f"b{name}")
                nc.sync.dma_start(btiles[name][:], bd[:])

            # q/k in [dh, t] layout: psum [128dh, 512t] -> split to per-head
            for name, dst in (("q", qh), ("k", kh)):
                for half in range(2):
                    hsl = slice(128 * half, 128 * (half + 1))
                    for chk in range(C // 512):
                        csl = slice(512 * chk, 512 * (chk + 1))
                        ps = ppsum.tile([128, 512], F32, tag="pqk", name="pqk")
                        nc.tensor.matmul(ps[:], btiles[name][:, hsl], ones[:],
                                         start=True, stop=False)
                        for ei in range(8):
                            nc.tensor.matmul(ps[:], wtiles[name][ei][:, hsl],
                                             xt_t[ei][:, csl],
                                             start=False, stop=(ei == 7))
                        eng0 = nc.vector if name == "q" else nc.scalar
                        if name == "q":
                            nc.vector.tensor_copy(dst[2 * half][0:64, csl], ps[0:64, :])
                            nc.vector.tensor_copy(dst[2 * half + 1][0:64, csl], ps[64:128, :])
                        else:
                            nc.scalar.activation(dst[2 * half][0:64, csl], ps[0:64, :], AFT.Copy)
                            nc.scalar.activation(dst[2 * half + 1][0:64, csl], ps[64:128, :], AFT.Copy)

            # v in [t, dh] layout: psum [128t, 256dh] -> spread into vt cols
            for ti in range(NQT):
                tsl = slice(128 * ti, 128 * (ti + 1))
                ps = ppsum.tile([128, DH], F32, tag="pv", name="pv")
                nc.tensor.matmul(ps[:], ones[:, :128], btiles["v"][:],
                                 start=True, stop=False)
                for ei in range(8):
                    nc.tensor.matmul(ps[:], xt_t[ei][:, tsl], wtiles["v"][ei][:],
                                     start=False, stop=(ei == 7))
                for h in range(HLOC):
                    nc.scalar.activation(vt[ti][:, 65 * h:65 * h + 64],
                                         ps[:, 64 * h:64 * h + 64], AFT.Copy)

        # ---- phase A pools ----
        u0_pool = ctx.enter_context(tc.tile_pool(name="u0", bufs=2))
        us_pool = ctx.enter_context(tc.tile_pool(name="us", bufs=4))
        dump_pool = ctx.enter_context(tc.tile_pool(name="dump", bufs=1))
        st_pool = ctx.enter_context(tc.tile_pool(name="st", bufs=2))
        ptk_pool = ctx.enter_context(tc.tile_pool(name="ptk", bufs=1))
        stu_pool = ctx.enter_context(tc.tile_pool(name="stu", bufs=3))
        misc_pool = ctx.enter_context(tc.tile_pool(name="misc", bufs=2))
        sc_psum = ctx.enter_context(tc.tile_pool(name="scp", bufs=3, space="PSUM"))
        tr_psum = ctx.enter_context(tc.tile_pool(name="trp", bufs=1, space="PSUM"))
        pv_psum = ctx.enter_context(tc.tile_pool(name="pvp", bufs=2, space="PSUM"))

        dumpD = dump_pool.tile([128, C], F16, tag="dumpD", name="dumpD")
        zz = dump_pool.tile([128, C], F16, tag="zz", name="zz")
        nc.vector.memset(zz[:], 0.0)

        def new_state(h):
            """Per-head Newton state tiles (pool rotates 2 buffers/tag)."""
            st = {}
            for nm in ("mx", "ntau0", "ntau", "dlt", "d1", "d2", "LtA", "LtD",
                       "QtD"):
                st[nm] = st_pool.tile([128, NQT], F32, tag=nm, name=f"{nm}_{h}")
            st["Lc"] = st_pool.tile([128, NQT], F32, tag="Lc", name=f"Lc_{h}")
            st["u0"] = [u0_pool.tile([128, 128 * (qi + 1)], F16, tag=f"u0_{qi}",
                                     name=f"u0_{qi}_{h}") for qi in range(NQT)]
            return st

        def emit_pass1_tile(h, st, qi):
            """Scores for one q-tile: matmul fp16, rowmax init, fused
            relu-copy to u0 fp16 with chunk-0 L accum."""
            ncol = 128 * (qi + 1)
            nchunk = (ncol + 511) // 512
            pss = []
            for ch in range(nchunk):
                c0 = 512 * ch
                w = min(512, ncol - c0)
                has_diag = (c0 + w == ncol)
                ps = sc_psum.tile([128, 512], F32, tag="sc", name="sc")
                pss.append((ps, c0, w))
                nc.tensor.matmul(ps[:, :w], qh[h][0:64, 128 * qi:128 * (qi + 1)],
                                 kh[h][0:64, c0:c0 + w],
                                 start=True, stop=not has_diag)
                if has_diag:
                    nc.tensor.matmul(ps[:, w - 128:w], ident16[:], mb[:],
                                     start=False, stop=True)
                if ch == 0:
                    # sampled rowmax -> ntau0 = 1 - max
                    nc.vector.reduce_max(st["mx"][:, qi:qi + 1],
                                         ps[:, 0:w:4], axis=AX.X)
                    nc.vector.tensor_scalar(out=st["ntau0"][:, qi:qi + 1],
                                            in0=st["mx"][:, qi:qi + 1],
                                            scalar1=1.0, scalar2=-1.0,
                                            op0=ALU.subtract, op1=ALU.mult)
            for ps, c0, w in pss:
                acc = {}
                if c0 == 0:
                    acc = dict(accum_out=st["Lc"][:, qi:qi + 1])
                nc.scalar.activation(st["u0"][qi][:, c0:c0 + w], ps[:, :w],
                                     AFT.Relu, bias=st["ntau0"][:, qi:qi + 1],
                                     **acc)

        def emit_passB(st, u0t, us, qi, w):
            """Q accum: p = (u0 + dlt) * u  (exact relu^2 on the support)."""
            nc.vector.scalar_tensor_tensor(
                out=dumpD[:, :w], in0=u0t[:, :w],
                scalar=st["dlt"][:, qi:qi + 1],
                in1=us[:, :w], op0=ALU.add, op1=ALU.mult,
                accum_out=st["QtD"][:, qi:qi + 1])

        def emit_update(h, st, div, frt, L_aps):
            """ntau = (ntau0 + dlt) - clamp((Q - frac) * 0.5 / L, -0.05, inf)

            L_aps: list of (slice, AP) covering the 16 tiles."""
            if frt is not None:
                nc.vector.tensor_tensor(out=st["d1"][:], in0=st["QtD"][:],
                                        in1=frt[:], op=ALU.subtract)
                nc.vector.tensor_scalar(out=st["d1"][:], in0=st["d1"][:],
                                        scalar1=0.5, scalar2=None, op0=ALU.mult)
            else:
                nc.vector.tensor_scalar(out=st["d1"][:], in0=st["QtD"][:],
                                        scalar1=1.0, scalar2=0.5,
                                        op0=ALU.subtract, op1=ALU.mult)
            for gs, lap in L_aps:
                nc.vector.tensor_scalar(out=st["d2"][:, gs], in0=lap,
                                        scalar1=1e-6, scalar2=None, op0=ALU.max)
            nc.vector.reciprocal(st["d2"][:], st["d2"][:])
            nc.vector.tensor_tensor(out=st["d1"][:], in0=st["d1"][:],
                                    in1=st["d2"][:], op=ALU.mult)
            nc.vector.tensor_scalar(out=st["d1"][:], in0=st["d1"][:],
                                    scalar1=-0.2, scalar2=None, op0=ALU.max)
            # ntau = (ntau0 + dlt) - d1   (dlt = clamped eval offset)
            nc.vector.tensor_tensor(out=st["d1"][:], in0=st["dlt"][:],
                                    in1=st["d1"][:], op=ALU.subtract)
            nc.vector.tensor_tensor(out=st["ntau"][:], in0=st["ntau0"][:],
                                    in1=st["d1"][:], op=ALU.add)

        def emit_first_update(h, st):
            """Update from the 512-col prefix: L from copy accums, Q via STT."""
            nc.vector.memset(st["dlt"][:], 0.0)
            for qi in range(NQT):
                w0 = min(512, 128 * (qi + 1))
                emit_passB(st, st["u0"][qi], st["u0"][qi], qi, w0)
            emit_update(h, st, 0, frc0, [(slice(0, NQT), st["Lc"][:])])

        def emit_sweep(h, st, div):
            # dlt = min(ntau - ntau0, 0); keeps masked u0 zeros inert
            nc.vector.tensor_tensor(out=st["dlt"][:], in0=st["ntau"][:],
                                    in1=st["ntau0"][:], op=ALU.subtract)
            nc.vector.tensor_scalar(out=st["dlt"][:], in0=st["dlt"][:],
                                    scalar1=0.0, scalar2=None, op0=ALU.min)
            k = ACT_START[div]
            for qi in range(NQT):
                ncol = 128 * (qi + 1)
                w = max(128, ncol // div)
                us = us_pool.tile([128, C], F16, tag="us", name="us")
                if qi >= k:
                    nc.scalar.activation(us[:, :w], st["u0"][qi][:, :w],
                                         AFT.Relu, bias=st["dlt"][:, qi:qi + 1],
                                         accum_out=st["LtA"][:, qi:qi + 1])
                else:
                    nc.vector.scalar_tensor_tensor(
                        out=us[:, :w], in0=st["u0"][qi][:, :w],
                        scalar=st["dlt"][:, qi:qi + 1], in1=zz[:, :w],
                        op0=ALU.add, op1=ALU.max,
                        accum_out=st["LtD"][:, qi:qi + 1])
                emit_passB(st, st["u0"][qi], us, qi, w)
            laps = [(slice(0, min(k, NQT)), st["LtD"][:, 0:min(k, NQT)])]
            if k < NQT:
                laps.append((slice(k, NQT), st["LtA"][:, k:NQT]))
            emit_update(h, st, div, fr.get(div), laps)

        def emit_st(h, st):
            """S^T recompute + p@v with normalizer fold + out scale."""
            # ntau -> qh[h] row 64 (transpose + row DMAs)
            ntp = tr_psum.tile([128, 512], F32, tag="tr", name="tr")
            nc.tensor.transpose(ntp[:NQT, :128], st["ntau"][:, :NQT], ident[:])
            ntT = misc_pool.tile([NQT, 128], F16, tag="ntT", name="ntT")
            nc.vector.tensor_copy(ntT[:], ntp[:NQT, :128])
            for qi in range(NQT):
                nc.sync.dma_start(qh[h][64:65, 128 * qi:128 * (qi + 1)],
                                  ntT[qi:qi + 1, :])
            # p^T per k-tile: K=65 matmul folds ntau; relu+square on ACT
            ptks = []
            for kt in range(NQT):
                q_lo = 128 * kt
                ptk = ptk_pool.tile([128, C - q_lo], F16, tag=f"ptk{kt}",
                                    name=f"ptk{kt}")
                ptks.append(ptk)
                for j in range(q_lo // 512, 4):
                    q0 = max(512 * j, q_lo)
                    w = 512 * (j + 1) - q0
                    has_diag = (q0 == q_lo)
                    ps = sc_psum.tile([128, 512], F32, tag="sc", name="sc")
                    nc.tensor.matmul(ps[:, :w], kh[h][:, q_lo:q_lo + 128],
                                     qh[h][:, q0:q0 + w],
                                     start=True, stop=not has_diag)
                    if has_diag:
                        nc.tensor.matmul(ps[:, :128], ident16[:], mbt[:],
                                         start=False, stop=True)
                    stu = stu_pool.tile([128, 512], F16, tag="stu", name="stu")
                    nc.scalar.activation(stu[:, :w], ps[:, :w], AFT.Relu)
                    nc.scalar.activation(ptk[:, q0 - q_lo:q0 - q_lo + w],
                                         stu[:, :w], AFT.Square)
            # p@v: po[0:64] = v^T p^T, po[64] = sum(p); scale by 1/S
            p2 = 64 * (h % 2)
            od = ohd[h // 2]
            for j in range(4):
                kts = [kt for kt in range(NQT) if 128 * kt < 512 * (j + 1)]
                po = pv_psum.tile([65, 512], F32, tag="po", name="po")
                for i, kt in enumerate(kts):
                    ak = max(512 * j, 128 * kt)
                    nc.tensor.matmul(
                        po[:, ak - 512 * j:512],
                        vt[kt][:, 65 * h:65 * h + 65],
                        ptks[kt][:, ak - 128 * kt:512 * (j + 1) - 128 * kt],
                        start=(i == 0), stop=(i == len(kts) - 1))
                rr32 = misc_pool.tile([1, 512], F32, tag="rr32", name="rr32")
                nc.vector.reciprocal(rr32[:], po[64:65, :])
                rr16 = misc_pool.tile([1, 512], F16, tag="rr16", name="rr16")
                nc.vector.tensor_copy(rr16[:], rr32[:])
                rb = tr_psum.tile([64, 512], F32, tag="rb", name="rb")
                nc.tensor.matmul(rb[:], ones[0:1, 0:64], rr16[:],
                                 start=True, stop=True)
                rbS = misc_pool.tile([64, 512], F16, tag="rbS", name="rbS")
                nc.scalar.activation(rbS[:], rb[:], AFT.Copy)
                nc.vector.tensor_tensor(
                    out=od[p2:p2 + 64, 512 * j:512 * (j + 1)],
                    in0=po[0:64, :], in1=rbS[:], op=ALU.mult)

        def emit_newton_pair(ha, hb):
            sts = {}
            for h in (ha, hb):
                sts[h] = new_state(h)
            for qi in range(NQT):
                for h in (ha, hb):
                    emit_pass1_tile(h, sts[h], qi)
            for h in (ha, hb):
                emit_first_update(h, sts[h])
            for div in SCHED:
                for h in (ha, hb):
                    emit_sweep(h, sts[h], div)
            return sts

        def emit_outproj(qis):
            for qi in qis:
                for chk in range(2):
                    csl = slice(512 * chk, 512 * (chk + 1))
                    ps = sc_psum.tile([128, 512], F32, tag="sc", name="sc")
                    for i in range(2):
                        nc.tensor.matmul(ps[:],
                                         ohd[i][:, 128 * qi:128 * (qi + 1)],
                                         wu_t[i][:, csl], start=(i == 0),
                                         stop=(i == 1))
                    osb = misc_pool.tile([128, 512], F32, tag="osb", name="osb")
                    nc.scalar.activation(osb[:], ps[:], AFT.Copy)
                    nc.sync.dma_start(out[128 * qi:128 * (qi + 1), csl], osb[:])

        # pipeline: Newton(0,1) -> ST(0) -> Newton(2,3) -> ST(1..3) -> O
        if DEBUG_TAPS:
            dbg_pool = ctx.enter_context(tc.tile_pool(name="dbgp", bufs=1))
        sts01 = emit_newton_pair(0, 1)
        if DEBUG_TAPS:
            st0 = sts01[0]
            f32c = dbg_pool.tile([128, C], F32, tag="dbgc", name="dbgc")
            nc.sync.dma_start(dbg_ntau[:], st0["ntau"][:])
            nc.vector.tensor_copy(f32c[:, :], st0["u0"][15][:, :])
            nc.sync.dma_start(dbg_u0[:], f32c[:])
            nc.sync.dma_start(dbg_lq[:, 0:NQT], st0["Lt"][:])
            nc.sync.dma_start(dbg_lq[:, NQT:2 * NQT], st0["QtD"][:])
            nc.sync.dma_start(dbg_lq[:, 2 * NQT:3 * NQT], st0["QtA"][:])
        emit_st(0, sts01[0])
        if DEBUG_TAPS:
            f32a = dbg_pool.tile([128, C], F32, tag="dbgc", name="dbga")
            nc.vector.tensor_copy(f32a[:65, :], qh[0][:, :])
            nc.sync.dma_start(dbg_qaug[:], f32a[:65, :])
        sts23 = emit_newton_pair(2, 3)
        emit_st(1, sts01[1])
        emit_st(2, sts23[2])
        emit_st(3, sts23[3])
        if DEBUG_TAPS:
            f32d = dbg_pool.tile([128, C], F32, tag="dbgc", name="dbgd")
            nc.vector.tensor_copy(f32d[:, :], ohd[0][:, :])
            nc.sync.dma_start(dbg_od[:], f32d[:])
        emit_outproj(range(NQT))

    nc.compile()
    return nc


def _get_nc():
    if "nc" not in _NC_CACHE:
        _NC_CACHE["nc"] = _build_nc()
    return _NC_CACHE["nc"]


def _entmax_bisect_np(X, alpha, n_iter=50):
    d = X.shape[-1]
    am1 = alpha - 1.0
    Xs = (X * am1).astype(np.float32)
    max_val = Xs.max(-1, keepdims=True)
    tau_lo = max_val - np.float32(1.0)
    tau_hi = max_val - np.float32((1.0 / d) ** (1.0 / am1))
    f_lo = (np.clip(Xs - tau_lo, 0, None) ** (1.0 / am1)).sum(-1, keepdims=True) - 1.0
    dm = tau_hi - tau_lo
    p_m = np.zeros_like(Xs)
    for _ in range(n_iter):
        dm = dm * 0.5
        tau_m = tau_lo + dm
        p_m = np.clip(Xs - tau_m, 0, None) ** (1.0 / am1)
        f_m = p_m.sum(-1, keepdims=True) - 1.0
        tau_lo = np.where(f_m * f_lo >= 0, tau_m, tau_lo)
    return p_m / p_m.sum(-1, keepdims=True)


def _numpy_fallback(x, mask, H, hs, alpha, Wq, bq, Wk, bk, Wv, bv, Wu, bu):
    b, c, e = x.shape
    q = (x @ Wq + bq).reshape(b, c, H, hs)
    k = (x @ Wk + bk).reshape(b, c, H, hs)
    v = (x @ Wv + bv).reshape(b, c, H, hs)
    dot = np.einsum('bqhd,bkhd->bhqk', q, k).astype(np.float32) / np.sqrt(hs)
    dot = np.where(mask[:, None], dot, -np.inf).astype(np.float32)
    p = _entmax_bisect_np(dot, float(alpha))
    o = np.einsum('bhqk,bkhd->bqhd', p, v).reshape(b, c, H * hs)
    return (o @ Wu + bu).astype(np.float32)


def make_in_maps(x, alpha_f, hs, Wq, bq, Wk, bk, Wv, bv, Wu):
    s = np.float32((alpha_f - 1.0) / np.sqrt(hs))  # fold entmax+attn scale into q
    mbias = np.triu(np.full((128, 128), NEG, np.float32), 1).astype(np.float16)
    mbiast = np.ascontiguousarray(mbias.T)
    # per-row prefix-subsample target fractions
    r = np.arange(128)[:, None]
    qi = np.arange(NQT)[None, :]
    valid = 128 * qi + r + 1
    frs = {}
    for dv in FRDIVS:
        w = np.maximum(128, (128 * (qi + 1)) // dv)
        frs[dv] = (np.minimum(w, valid) / valid).astype(np.float32)
    frc0v = (np.minimum(np.minimum(512, 128 * (qi + 1)), valid) / valid
             ).astype(np.float32)
    in_maps = []
    for core in range(8):
        bb = core // 4
        hsl = slice((core % 4) * DH, (core % 4) * DH + DH)
        im = {
            "xt": np.ascontiguousarray(x[bb].T).astype(np.float16),
            "wq": (np.ascontiguousarray(Wq[:, hsl]) * s).astype(np.float16),
            "bq": (bq[hsl] * s).reshape(1, DH).astype(np.float16),
            "wk": np.ascontiguousarray(Wk[:, hsl]).astype(np.float16),
            "bk": bk[hsl].reshape(1, DH).astype(np.float16),
            "wv": np.ascontiguousarray(Wv[:, hsl]).astype(np.float16),
            "bv": bv[hsl].reshape(1, DH).astype(np.float16),
            "wu": np.ascontiguousarray(Wu[hsl, :]).astype(np.float16),
            "mb": mbias,
            "mbt": mbiast,
        }
        for dv in FRDIVS:
            im[f"fr{dv}"] = frs[dv]
        im["frc0"] = frc0v
        in_maps.append(im)
    return in_maps


def kernel(x, attention_mask, num_heads, head_size, alpha,
           Wq, bq, Wk, bk, Wv, bv, Wu, bu):
    x = np.asarray(x, np.float32)
    mask = np.asarray(attention_mask)
    H = int(num_heads)
    hs = int(head_size)
    alpha_f = float(np.asarray(alpha))
    Wq = np.asarray(Wq, np.float32); bq = np.asarray(bq, np.float32)
    Wk = np.asarray(Wk, np.float32); bk = np.asarray(bk, np.float32)
    Wv = np.asarray(Wv, np.float32); bv = np.asarray(bv, np.float32)
    Wu = np.asarray(Wu, np.float32); bu = np.asarray(bu, np.float32)
    b, c, e = x.shape

    causal = np.tril(np.ones((c, c), dtype=bool))
    supported = (
        (b, c, e, H, hs) == (2, C, E, 16, HS)
        and abs(alpha_f - 1.5) < 1e-6
        and all(np.array_equal(mask[i], causal) for i in range(b))
    )
    if not supported:
        return _numpy_fallback(x, mask, H, hs, alpha_f,
                               Wq, bq, Wk, bk, Wv, bv, Wu, bu)

    from concourse.bass_utils import run_bass_kernel_spmd

    nc = _get_nc()
    in_maps = make_in_maps(x, alpha_f, hs, Wq, bq, Wk, bk, Wv, bv, Wu)
    res = run_bass_kernel_spmd(nc, in_maps, core_ids=list(range(8)))
    o = [res.results[i]["out"] for i in range(8)]
    full = np.stack([o[0] + o[1] + o[2] + o[3],
                     o[4] + o[5] + o[6] + o[7]]) + bu
    return full.astype(np.float32)
